# revision 29
# baseline (speedup 1.0000x reference)
"""BinaryLeNet5 forward on 8 TRN2 NeuronCores, pure data parallel (1024 imgs/core).

v2: conv1 is x-stationary. Per 128-image block, the stationary operand is the
image slab [97 = (c,wi)+bias-row, 128 imgs] for one input row h; the moving
operand is the banded-Toeplitz weight slice [97, 168 = par*wo2*o] for one kh
tap. This uses the full 128 PE columns (vs 84 for the T-stationary form) and
folds both wo-parities into one stream, cutting conv1 PE cycles per image per
pass from 280 to 183.75. The fp16 hi/lo precision ladder is unchanged
(~22 effective mantissa bits, bit-exact vs f32 reference in practice). The
conv1 bias rides as a 97th contraction row (ones in x-hi, b1 in the kh=0
Toeplitz columns), so PSUM holds conv+bias and pooling/sign need no bias.

PSUM accumulation groups are bank-granular (2KB zero regions): a chunk bank
is started only by its first matmul (even ho, hi, kh=0) - the odd-ho half
overwrites its pending-zero bytes on first touch - and stopped only by its
last (odd ho, lo, kh=4). PSUM (8 banks): 5x conv1 ho-pair chunks [128, 336]
f32 (1 bank each), 1x transpose rotator [84, 8*128] bf16, 2x conv2/fc
[80, 512] f32 (1 bank each, bufs=2 so phases pipeline).

Per (tile, ho2): DVE 4-way max (ho-pair in chunk col-slots, wo-parity in par
halves) -> bf16 [128, 84]; PE transpose (identity matmul) -> [84, 128] psum;
ACT sign reads it once and writes the fp8 result to both x2 slots (slot-minor
x2 layout makes the two destinations an adjacent stride-N pair). Pools run
two h-steps after their chunk completes so the in-order PE never waits.

conv2: kh taps {0,1},{2,3} as fp8 DoubleRow matmuls + tap 4 plain, in four
1-bank phases per pooling pair (hop x wop): A-phases stage to SBUF via ACT
copies, B-phases pool against the stage (3 DVE tensor_max), then sign+bias.
fc1/fc2/fc3 unchanged (DR fp8 / bf16).

DMA: t1h + xh tiles on the sync HWDGE ring, t1l + xl on the scalar ring (a
matmul's single wait slot transitively covers same-queue weights). Each tile
is a [96, 4096] chunk (96 % 16 == 0 so descriptors spray across all 16 SDMA
engines; 97 rows would land on ONE engine) plus a tiny constant-row DMA.
conv2/fc weights+biases ride the GPSIMD SWDGE ring. conv2/fc for the first
512 images interleave into tiles 4-6; the second half's blocks interleave
into tile 7 as its pooled rows appear, leaving only c=4 + fc as the tail.
"""

import os
import sys

import numpy as np

sys.path.insert(0, "/opt/trn_rl_repo")

import ml_dtypes  # noqa: E402

BF16 = ml_dtypes.bfloat16
F8E4 = ml_dtypes.float8_e4m3

B = 8192
NCORES = 8
N = B // NCORES  # 1024 images per core
NT = 8  # image tiles per core
TI = N // NT  # 128 images per tile
NBLK = 2  # conv2/fc blocks of 512 columns
NB = N // NBLK  # 512


def _binarize(w):
    return np.where(w >= 0, 1.0, -1.0).astype(np.float32)


def _build_t1h(w1, b1):
    # t1[c*32+wi, kh*168 + par*84 + wo2*6 + o] = w1b[o,c,kh,kw]
    #   wo = 2*wo2 + par, kw = wi - wo, valid 0<=kw<5
    # row 96: bias row = b1[o] on kh=0 columns (x-hi row 96 is all ones)
    w1b = _binarize(w1)  # [6,3,5,5]
    t1 = np.zeros((97, 5 * 168), np.float32)
    for kh in range(5):
        for par in range(2):
            for wo2 in range(14):
                wo = 2 * wo2 + par
                for o in range(6):
                    col = (4 - kh) * 168 + par * 84 + wo2 * 6 + o
                    for c in range(3):
                        for kw in range(5):
                            wi = wo + kw
                            if wi < 32:
                                t1[c * 32 + wi, col] = w1b[o, c, kh, kw]
                    if kh == 0:
                        t1[96, col] = b1[o]
    return t1.astype(np.float16)


def _build_t1l(w1, b1):
    # lo-pass weights: the fp16 Toeplitz scaled by 2^-11 (exact fp16 normals).
    # Row 96 on kh=0 columns carries the bias residual (b1 - fp16(b1)) * 2^11
    # so the bias gets the same ~22-bit ladder as the data (x-lo row 96 = 1).
    w1b = _binarize(w1)
    b1r = (b1.astype(np.float32) - b1.astype(np.float16).astype(np.float32)) * 2048.0
    t1 = np.zeros((97, 5 * 168), np.float32)
    for kh in range(5):
        for par in range(2):
            for wo2 in range(14):
                wo = 2 * wo2 + par
                for o in range(6):
                    col = (4 - kh) * 168 + par * 84 + wo2 * 6 + o
                    for c in range(3):
                        for kw in range(5):
                            wi = wo + kw
                            if wi < 32:
                                t1[c * 32 + wi, col] = w1b[o, c, kh, kw] * 2.0**-11
                    if kh == 0:
                        # x-lo row 96 is 2^-11, so this lands as b1r * 2^-11
                        # = b1 - fp16(b1) with all-normal fp16 operands.
                        t1[96, col] = b1r[o]
    return t1.astype(np.float16)


def _build_t2(w2):
    # DR pairs: t2dr[w2*6+c, s, (ks*2+wop)*96 + wo2*16+o] = w2b[o,c,2ks+s,kw]
    # tap4:     t24 [w2*6+c, wop*96 + wo2*16+o] = w2b[o,c,4,kw]
    w2b = _binarize(w2)  # [16,6,5,5]
    t2dr = np.zeros((84, 2, 4 * 96), np.float32)
    t24 = np.zeros((84, 2 * 96), np.float32)
    for wop in range(2):
        for wo2 in range(5):
            wo = 2 * wo2 + wop
            for o in range(16):
                for c in range(6):
                    for kw in range(5):
                        w2i = wo + kw
                        if w2i >= 14:
                            continue
                        row = w2i * 6 + c
                        for ks in range(2):
                            for s in range(2):
                                t2dr[row, s, (ks * 2 + wop) * 96 + wo2 * 16 + o] = w2b[
                                    o, c, 2 * ks + s, kw
                                ]
                        t24[row, wop * 96 + wo2 * 16 + o] = w2b[o, c, 4, kw]
    return (
        np.ascontiguousarray(t2dr.reshape(84, 2 * 384)).astype(F8E4),
        t24.astype(F8E4),
    )


def _build_f1(wf1):
    # DR pairs: f1dr[w*16+o, s, h5p*128 + f] = wf1b[f, o*25+(2*h5p+s)*5+w]
    # tap4:     f14 [w*16+o, f] = wf1b[f, o*25+20+w]
    wf1b = _binarize(wf1)  # [120, 400]
    f1dr = np.zeros((80, 2, 2 * 128), np.float32)
    f14 = np.zeros((80, 120), np.float32)
    for w in range(5):
        for o in range(16):
            row = w * 16 + o
            for h5p in range(2):
                for s in range(2):
                    f1dr[row, s, h5p * 128 : h5p * 128 + 120] = wf1b[
                        :, o * 25 + (2 * h5p + s) * 5 + w
                    ]
            f14[row, :] = wf1b[:, o * 25 + 20 + w]
    return (
        np.ascontiguousarray(f1dr.reshape(80, 2 * 256)).astype(F8E4),
        f14.astype(F8E4),
    )


_CACHE = {}


def _get_nc():
    if "nc" in _CACHE:
        return _CACHE["nc"]
    import concourse.bacc as bacc
    import concourse.mybir as mybir
    import concourse.tile as tile

    f32 = mybir.dt.float32
    f16 = mybir.dt.float16
    bf16 = mybir.dt.bfloat16
    f8e4 = mybir.dt.float8e4
    DR = mybir.MatmulPerfMode.DoubleRow

    nc = bacc.Bacc()
    xh_d = nc.dram_tensor("xh", [97, 32 * N], f16, kind="ExternalInput")
    xl_d = nc.dram_tensor("xl", [97, 32 * N], f16, kind="ExternalInput")
    t1h_d = nc.dram_tensor("t1h", [97, 840], f16, kind="ExternalInput")
    t1l_d = nc.dram_tensor("t1l", [97, 840], f16, kind="ExternalInput")
    id_d = nc.dram_tensor("idm", [128, 128], bf16, kind="ExternalInput")
    t2dr_d = nc.dram_tensor("t2dr", [84, 768], f8e4, kind="ExternalInput")
    t24_d = nc.dram_tensor("t24", [84, 192], f8e4, kind="ExternalInput")
    f1dr_d = nc.dram_tensor("f1dr", [80, 512], f8e4, kind="ExternalInput")
    f14_d = nc.dram_tensor("f14", [80, 120], f8e4, kind="ExternalInput")
    f2_d = nc.dram_tensor("f2", [120, 84], bf16, kind="ExternalInput")
    f3_d = nc.dram_tensor("f3", [84, 10], bf16, kind="ExternalInput")
    b2_d = nc.dram_tensor("b2v", [80, 1], f32, kind="ExternalInput")
    bf1_d = nc.dram_tensor("bf1v", [120, 1], f32, kind="ExternalInput")
    bf2_d = nc.dram_tensor("bf2v", [84, 1], f32, kind="ExternalInput")
    bf3_d = nc.dram_tensor("bf3v", [10, 1], f32, kind="ExternalInput")
    out_d = nc.dram_tensor("out", [10, N], f32, kind="ExternalOutput")

    with tile.TileContext(nc) as tc:
        with (
            tc.tile_pool(name="xtp", bufs=1) as xtp,
            tc.tile_pool(name="wts", bufs=1) as wts,
            tc.tile_pool(name="acts", bufs=1) as acts,
            tc.tile_pool(name="ev", bufs=4) as ev,
            tc.tile_pool(name="ps", bufs=1, space="PSUM") as ps,
        ):
            # ---- DMA. sync ring: t1h then the xh tiles; scalar ring: t1l
            # then the xl tiles. A matmul's single wait slot on its x tile
            # transitively covers the same-queue weight DMAs. Each x tile is
            # two DMAs: rows 0:96 (96 divides by 16 -> descriptors spray
            # across all 16 SDMA engines; a 97-row DMA lands on ONE engine)
            # plus the constant ladder row 96 as a tiny flat transfer.
            # Readers wait per-TILE, so tile 0 is ready ~8us in instead of
            # after the whole 12.6MB stream.
            t1hs = wts.tile([97, 840], f16, tag="t1h")
            nc.sync.dma_start(out=t1hs[0:96, :], in_=t1h_d[0:96, :])
            nc.sync.dma_start(out=t1hs[96:97, :], in_=t1h_d[96:97, :])
            t1ls = wts.tile([97, 840], f16, tag="t1l")
            nc.scalar.dma_start(out=t1ls[0:96, :], in_=t1l_d[0:96, :])
            nc.scalar.dma_start(out=t1ls[96:97, :], in_=t1l_d[96:97, :])

            ids = wts.tile([128, 128], bf16, tag="idm")
            xhs, xls = [], []
            for t in range(NT):
                xhs.append(xtp.tile([97, 32 * TI], f16, tag=f"xh{t}", name=f"xh{t}"))
                xls.append(xtp.tile([97, 32 * TI], f16, tag=f"xl{t}", name=f"xl{t}"))
            for t in range(NT):
                sl = slice(t * 32 * TI, (t + 1) * 32 * TI)
                nc.sync.dma_start(out=xhs[t][96:97, :], in_=xh_d[96:97, sl])
                nc.sync.dma_start(out=xhs[t][0:96, :], in_=xh_d[0:96, sl])
                nc.scalar.dma_start(out=xls[t][96:97, :], in_=xl_d[96:97, sl])
                nc.scalar.dma_start(out=xls[t][0:96, :], in_=xl_d[0:96, sl])
                if t == 0:
                    # identity for the transposes - needed from tile 0's
                    # first pool (~14us in), after tile 0's data
                    nc.sync.dma_start(out=ids, in_=id_d[:, :])

            # conv2/fc weights+biases ride the GPSIMD SWDGE ring.
            t2drs = wts.tile([84, 768], f8e4, tag="t2dr")
            nc.gpsimd.dma_start(out=t2drs, in_=t2dr_d[:, :])
            t24s = wts.tile([84, 192], f8e4, tag="t24")
            nc.gpsimd.dma_start(out=t24s, in_=t24_d[:, :])
            f1drs = wts.tile([80, 512], f8e4, tag="f1dr")
            nc.gpsimd.dma_start(out=f1drs, in_=f1dr_d[:, :])
            f14s = wts.tile([80, 120], f8e4, tag="f14")
            nc.gpsimd.dma_start(out=f14s, in_=f14_d[:, :])
            f2s = wts.tile([120, 84], bf16, tag="f2")
            nc.gpsimd.dma_start(out=f2s, in_=f2_d[:, :])
            f3s = wts.tile([84, 10], bf16, tag="f3")
            nc.gpsimd.dma_start(out=f3s, in_=f3_d[:, :])
            b2s = wts.tile([80, 1], f32, tag="b2")
            nc.gpsimd.dma_start(out=b2s, in_=b2_d[:, :])
            bf1s = wts.tile([120, 1], f32, tag="bf1")
            nc.gpsimd.dma_start(out=bf1s, in_=bf1_d[:, :])
            bf2s = wts.tile([84, 1], f32, tag="bf2")
            nc.gpsimd.dma_start(out=bf2s, in_=bf2_d[:, :])
            bf3s = wts.tile([10, 1], f32, tag="bf3")
            nc.gpsimd.dma_start(out=bf3s, in_=bf3_d[:, :])

            # One consumer-engine 'touch' per DMA'd bias tile (carries the DMA
            # wait so later ops on that engine need no extra wait slot).
            tb2 = wts.tile([80, 1], f32, tag="tb2")
            nc.scalar.copy(tb2, b2s)
            tb3 = wts.tile([120, 1], f32, tag="tb3")
            nc.scalar.copy(tb3, bf1s)
            tb4 = wts.tile([84, 1], f32, tag="tb4")
            nc.scalar.copy(tb4, bf2s)
            tb5 = wts.tile([10, 1], f32, tag="tb5")
            nc.vector.tensor_copy(tb5, bf3s)

            # ---- activations / layouts
            # x2: slot-minor [84, a=2*r+s, n]; the rhs DR pair for conv2 row r
            # is columns (2r, 2r+1) = (pooled r, pooled r+1); pooled row q is
            # written to a = 2q (s0) and a = 2q-1 (s1), an adjacent pair.
            x2dr = acts.tile([84, 28 * N], f8e4, tag="x2dr")
            x2b = x2dr.rearrange("p (a n) -> p a n", a=28)
            x3dr = acts.tile([80, 2 * 3 * N], f8e4, tag="x3dr")
            x3v = x3dr.rearrange("p (s f) -> p s f", s=2)
            x4 = acts.tile([120, N], bf16, tag="x4")
            x5 = acts.tile([84, N], bf16, tag="x5")
            outs = acts.tile([10, N], f32, tag="outs")
            c2stage = acts.tile([80, 2 * NB], f32, tag="c2s")
            f1drv = f1drs.rearrange("p (s f) -> p s f", s=2)

            # PSUM: 5 conv1 chunks (1 bank) + transpose rotator (1) + c2 (2).
            NTP = 8
            tp = ps.tile([84, NTP * 128], bf16, tag="tp", name="tp")

            def conv1_groups(h):
                # t1 columns are kh-DESCENDING (block b holds kh=4-b), so the
                # two middle taps whose ho's form one chunk's (even, odd)
                # pair are a single contiguous 336-col matmul writing the
                # whole chunk. Those taps are always accumulation-middle
                # (never kh=0-start / kh=4-stop), so flags stay on singles.
                kset = [kh for kh in range(5) if 0 <= h - kh <= 27]
                pair = (2, 1) if h % 2 == 0 else (3, 2)
                groups = []
                if pair[0] in kset and pair[1] in kset:
                    b0 = 4 - pair[0]
                    groups.append((slice(b0 * 168, (b0 + 2) * 168),
                                   (h - pair[0]) >> 1, slice(0, 336), None))
                    kset = [kh for kh in kset if kh not in pair]
                for kh in kset:
                    ho = h - kh
                    b = 4 - kh
                    groups.append((slice(b * 168, (b + 1) * 168), ho >> 1,
                                   slice((ho & 1) * 168, (ho & 1) * 168 + 168), kh))
                return groups

            def conv1_h(t, h, chunks, passes=("hi", "lo")):
                # PSUM accumulation groups are bank-granular (2KB zero
                # regions): only the bank's FIRST matmul (even ho, hi pass,
                # kh=0) starts it - the odd-ho half then overwrites its
                # pending-zero bytes on first touch - and only the bank's
                # LAST matmul (odd ho, lo pass, kh=4) stops it.
                lhs_h = xhs[t][:, h * TI : (h + 1) * TI]
                lhs_l = xls[t][:, h * TI : (h + 1) * TI]
                for p in passes:
                    for wsl, ci, osl, kh in conv1_groups(h):
                        if p == "hi":
                            nc.tensor.matmul(
                                chunks[ci][:, osl], lhs_h, t1hs[:, wsl],
                                start=(kh == 0 and (h & 1) == 0), stop=False,
                            )
                        else:
                            nc.tensor.matmul(
                                chunks[ci][:, osl], lhs_l, t1ls[:, wsl],
                                start=False, stop=(kh == 4 and (h & 1) == 1),
                            )

            def pool_sign(t, ho2, chunks):
                # 4-way max: ho-pair = chunk col-halves (slots), wo-pair =
                # par halves within a slot -> [128, 84, (s p)] reduce X.
                e = ev.tile([128, 84], bf16, tag="e", bufs=16, name="e")
                nc.vector.reduce_max(
                    e,
                    chunks[ho2].rearrange("q (s p n) -> q n (s p)", s=2, p=2),
                    axis=mybir.AxisListType.X,
                )
                tps = tp[:, (ho2 % NTP) * 128 : (ho2 % NTP) * 128 + 128]
                nc.tensor.transpose(tps, e, ids)
                lo = max(0, 2 * ho2 - 1)
                outp = x2b[:, lo : 2 * ho2 + 1, t * TI : (t + 1) * TI]
                inp = tps.unsqueeze(1)
                if 2 * ho2 + 1 - lo == 2:
                    inp = inp.broadcast_to([84, 2, TI])
                nc.scalar.sign(outp, inp)

            def conv2_phase(c, nb, hop, wop):
                # one conv2 output row hb = 2c+hop, one wo parity, NB imgs;
                # [80, NB] f32 = exactly one PSUM bank; bufs=2 so the next
                # phase's matmuls overlap this phase's stage-copy/pools.
                hb = 2 * c + hop
                nbsl = slice(nb * NB, nb * NB + NB)
                c2t = ps.tile([80, NB], f32, tag="c2", bufs=2, name="c2t")
                for ks in range(2):
                    lhs = t2drs.rearrange("p (s f) -> p s f", s=2)[
                        :, :, (ks * 2 + wop) * 96 : (ks * 2 + wop) * 96 + 80
                    ]
                    r = hb + 2 * ks
                    nc.tensor.matmul(
                        c2t, lhs, x2b[:, 2 * r : 2 * r + 2, nbsl],
                        start=(ks == 0), stop=False, perf_mode=DR,
                    )
                nc.tensor.matmul(
                    c2t, t24s[:, wop * 96 : wop * 96 + 80],
                    x2b[:, 2 * (hb + 4), nbsl], start=False, stop=True,
                )
                return c2t

            def conv2_a(c, nb):
                for wop in range(2):
                    c2t = conv2_phase(c, nb, 0, wop)
                    nc.scalar.copy(c2stage[:, wop * NB : wop * NB + NB], c2t)

            def conv2_b(c, nb):
                ms = []
                for wop in range(2):
                    c2t = conv2_phase(c, nb, 1, wop)
                    m = ev.tile([80, NB], f32, tag="ev", name="m")
                    nc.vector.tensor_max(
                        m, c2t, c2stage[:, wop * NB : wop * NB + NB]
                    )
                    ms.append(m)
                e2 = ev.tile([80, NB], f32, tag="ev", name="e2")
                nc.vector.tensor_max(e2, ms[0], ms[1])
                nc.scalar.sign(
                    x3v[:, c % 2, (c // 2) * N + nb * NB : (c // 2) * N + nb * NB + NB],
                    e2, bias=b2s,
                )

            def fc_block(nb, b0=None, w=None):
                if b0 is None:
                    b0, w = nb * NB, NB
                nbsl = slice(b0, b0 + w)
                p3 = ps.tile([120, w], f32, tag="c2", bufs=2, name="p3")
                for h5p in range(2):
                    nc.tensor.matmul(
                        p3, f1drv[:, :, h5p * 128 : h5p * 128 + 120],
                        x3v[:, :, h5p * N + b0 : h5p * N + b0 + w],
                        start=(h5p == 0), stop=False, perf_mode=DR,
                    )
                nc.tensor.matmul(
                    p3, f14s, x3v[:, 0, 2 * N + b0 : 2 * N + b0 + w],
                    start=False, stop=True,
                )
                nc.scalar.sign(x4[:, nbsl], p3, bias=bf1s)

                p4 = ps.tile([84, w], f32, tag="c2", bufs=2, name="p4")
                nc.tensor.matmul(p4, f2s, x4[:, nbsl], start=True, stop=True)
                nc.scalar.sign(x5[:, nbsl], p4, bias=bf2s)

                p5 = ps.tile([10, w], f32, tag="c2", bufs=2, name="p5")
                nc.tensor.matmul(p5, f3s, x5[:, nbsl], start=True, stop=True)
                nc.vector.tensor_scalar_add(outs[:, nbsl], p5, bf3s)
                nc.sync.dma_start(out=out_d[:, nbsl], in_=outs[:, nbsl])

            # conv2/fc interleave schedule: (t, h) -> list of emitter thunks.
            # nb=0 spreads through tiles 4-6, nb=1 through tile 7 as its
            # pooled rows appear; nb=1's last block + fc trail at the end.
            def emit_a(c, nb):
                return lambda: conv2_a(c, nb)

            def emit_b(c, nb):
                return lambda: conv2_b(c, nb)

            sched = {}
            for c in range(5):
                sched.setdefault((4 + (c // 2), (4 * c) % 8 + 2), []).append(emit_a(c, 0))
                sched.setdefault((4 + (c // 2), (4 * c) % 8 + 4), []).append(emit_b(c, 0))
            sched.setdefault((6, 24), []).append(lambda: fc_block(0))
            for c in range(4):
                sched.setdefault((7, 16 + 4 * c), []).append(emit_a(c, 1))
                sched.setdefault((7, 18 + 4 * c), []).append(emit_b(c, 1))

            # pool chunk a two h-steps after its last matmul (h = 2a+7) so
            # the in-order PE doesn't stall at the transpose waiting on DVE;
            # ho2=13's pool carries into the next tile's h=1 slot.
            prev_carry = []
            for t in range(NT):
                chunks = {}
                carry, prev_carry = prev_carry, []
                for h in range(32):
                    if h % 2 == 0 and h <= 26:
                        chunks[h >> 1] = ps.tile(
                            [128, 336], f32, tag="chunk", bufs=5, name="ck"
                        )
                    if t == 0 and h < 3:
                        # tile 0 warmup: hi-pass only while the xl stream is
                        # still landing; the lo passes backfill before h+3's
                        # groups (so each chunk's stop stays its last write)
                        conv1_h(t, h, chunks, passes=("hi",))
                    else:
                        if t == 0 and h < 6:
                            conv1_h(t, h - 3, chunks, passes=("lo",))
                        conv1_h(t, h, chunks)
                    if h == 1 and carry:
                        carry.pop(0)()
                    if h >= 7 and h % 2 == 1 and h <= 31:
                        a = (h - 7) // 2
                        pool_sign(t, a, chunks)
                    for fn in sched.get((t, h), []):
                        fn()
                prev_carry.append(
                    (lambda tt, ck: lambda: pool_sign(tt, 13, ck))(t, chunks)
                )
            for fn in prev_carry:  # tile 7's last pool
                fn()
            # tail: last conv2 block of nb=1, then fc
            sched_tail = [emit_a(4, 1), emit_b(4, 1)]
            for fn in sched_tail:
                fn()
            fc_block(1, NB, NB // 2)
            fc_block(1, NB + NB // 2, NB // 2)

    nc.finalize()
    _CACHE["nc"] = nc
    return nc


def _install_ntff_hook():
    """The container's antenv stub lacks axon_hooks; synthesize it and register
    the ctypes-based NTFF profile hook from the axon boot module."""
    if "hook" in _CACHE:
        return
    _CACHE["hook"] = True
    try:
        import types
        import antenv

        if not hasattr(antenv, "axon_hooks"):
            store = {"h": None}
            m = types.ModuleType("antenv.axon_hooks")
            m.set_axon_ntff_profile_hook = lambda h: store.update(h=h)
            m.get_axon_ntff_profile_hook = lambda: store["h"]
            sys.modules["antenv.axon_hooks"] = m
            antenv.axon_hooks = m
            sys.path.insert(0, "/root/.axon_site")
            from trn_agent_boot.trn_boot import _ntff_profile_via_ctypes

            m.set_axon_ntff_profile_hook(
                _ntff_profile_via_ctypes("/opt/axon/libaxon_pjrt.so")
            )
    except Exception as e:  # profiling is best-effort
        print(f"ntff hook install failed: {e}", file=sys.stderr)


def kernel(x, w1, b1, w2, b2, wf1, bf1, wf2, bf2, wf3, bf3):
    nc = _get_nc()
    _install_ntff_hook()
    from concourse import bass_utils

    # host-side relayout: xt[core][c*32+w, (t, h, n)] = x[core*N + t*TI + n, c, h, w]
    xr = np.ascontiguousarray(
        x.reshape(NCORES, NT, TI, 3, 32, 32).transpose(0, 3, 5, 1, 4, 2)
    )  # [cores, 3, 32(w), NT, 32(h), TI]

    xh = np.ones((NCORES, 97, NT, 32, TI), np.float16)
    xh[:, :96] = xr.reshape(NCORES, 96, NT, 32, TI).astype(np.float16)
    xl = np.full((NCORES, 97, NT, 32, TI), 2.0**-11, np.float16)
    xl[:, :96] = (
        (xr.reshape(NCORES, 96, NT, 32, TI) - xh[:, :96].astype(np.float32)) * 2048.0
    ).astype(np.float16)

    t2dr, t24 = _build_t2(w2)
    f1dr, f14 = _build_f1(wf1)
    shared = {
        "t1h": _build_t1h(w1, b1),
        "t1l": _build_t1l(w1, b1),
        "idm": np.eye(128, dtype=BF16),
        "t2dr": t2dr, "t24": t24, "f1dr": f1dr, "f14": f14,
        "f2": np.ascontiguousarray(_binarize(wf2).T).astype(BF16),
        "f3": np.ascontiguousarray(_binarize(wf3).T).astype(BF16),
        "b2v": np.tile(b2.astype(np.float32), 5).reshape(80, 1),
        "bf1v": bf1.astype(np.float32).reshape(120, 1),
        "bf2v": bf2.astype(np.float32).reshape(84, 1),
        "bf3v": bf3.astype(np.float32).reshape(10, 1),
    }
    in_maps = [
        dict(
            shared,
            xh=np.ascontiguousarray(xh[i].reshape(97, 32 * N)),
            xl=np.ascontiguousarray(xl[i].reshape(97, 32 * N)),
        )
        for i in range(NCORES)
    ]

    res = bass_utils.run_bass_kernel_spmd(
        nc, in_maps, core_ids=list(range(NCORES)),
        trace=bool(int(os.environ.get("KERNEL_TRACE", "0"))),
    )
    if res.exec_time_ns is not None:
        print(f"HW exec time: {res.exec_time_ns} ns")
    out = np.stack([r["out"] for r in res.results])  # [8, 10, N]
    return np.ascontiguousarray(out.transpose(0, 2, 1)).reshape(B, 10).astype(np.float32)


# revision 30
# speedup vs baseline: 1.0054x; 1.0054x over previous
"""BinaryLeNet5 forward on 8 TRN2 NeuronCores, pure data parallel (1024 imgs/core).

v2: conv1 is x-stationary. Per 128-image block, the stationary operand is the
image slab [97 = (c,wi)+bias-row, 128 imgs] for one input row h; the moving
operand is the banded-Toeplitz weight slice [97, 168 = par*wo2*o] for one kh
tap. This uses the full 128 PE columns (vs 84 for the T-stationary form) and
folds both wo-parities into one stream, cutting conv1 PE cycles per image per
pass from 280 to 183.75. The fp16 hi/lo precision ladder is unchanged
(~22 effective mantissa bits, bit-exact vs f32 reference in practice). The
conv1 bias rides as a 97th contraction row (ones in x-hi, b1 in the kh=0
Toeplitz columns), so PSUM holds conv+bias and pooling/sign need no bias.

PSUM accumulation groups are bank-granular (2KB zero regions): a chunk bank
is started only by its first matmul (even ho, hi, kh=0) - the odd-ho half
overwrites its pending-zero bytes on first touch - and stopped only by its
last (odd ho, lo, kh=4). PSUM (8 banks): 5x conv1 ho-pair chunks [128, 336]
f32 (1 bank each), 1x transpose rotator [84, 8*128] bf16, 2x conv2/fc
[80, 512] f32 (1 bank each, bufs=2 so phases pipeline).

Per (tile, ho2): DVE 4-way max (ho-pair in chunk col-slots, wo-parity in par
halves) -> bf16 [128, 84]; PE transpose (identity matmul) -> [84, 128] psum;
ACT sign reads it once and writes the fp8 result to both x2 slots (slot-minor
x2 layout makes the two destinations an adjacent stride-N pair). Pools run
two h-steps after their chunk completes so the in-order PE never waits.

conv2: kh taps {0,1},{2,3} as fp8 DoubleRow matmuls + tap 4 plain, in four
1-bank phases per pooling pair (hop x wop): A-phases stage to SBUF via ACT
copies, B-phases pool against the stage (3 DVE tensor_max), then sign+bias.
fc1/fc2/fc3 unchanged (DR fp8 / bf16).

DMA: t1h + xh tiles on the sync HWDGE ring, t1l + xl on the scalar ring (a
matmul's single wait slot transitively covers same-queue weights). Each tile
is a [96, 4096] chunk (96 % 16 == 0 so descriptors spray across all 16 SDMA
engines; 97 rows would land on ONE engine) plus a tiny constant-row DMA.
conv2/fc weights+biases ride the GPSIMD SWDGE ring. conv2/fc for the first
512 images interleave into tiles 4-6; the second half's blocks interleave
into tile 7 as its pooled rows appear, leaving only c=4 + fc as the tail.
"""

import os
import sys

import numpy as np

sys.path.insert(0, "/opt/trn_rl_repo")

import ml_dtypes  # noqa: E402

BF16 = ml_dtypes.bfloat16
F8E4 = ml_dtypes.float8_e4m3

B = 8192
NCORES = 8
N = B // NCORES  # 1024 images per core
NT = 8  # image tiles per core
TI = N // NT  # 128 images per tile
NBLK = 2  # conv2/fc blocks of 512 columns
NB = N // NBLK  # 512


def _binarize(w):
    return np.where(w >= 0, 1.0, -1.0).astype(np.float32)


def _build_t1h(w1, b1):
    # t1[c*32+wi, kh*168 + par*84 + wo2*6 + o] = w1b[o,c,kh,kw]
    #   wo = 2*wo2 + par, kw = wi - wo, valid 0<=kw<5
    # row 96: bias row = b1[o] on kh=0 columns (x-hi row 96 is all ones)
    w1b = _binarize(w1)  # [6,3,5,5]
    t1 = np.zeros((97, 5 * 168), np.float32)
    for kh in range(5):
        for par in range(2):
            for wo2 in range(14):
                wo = 2 * wo2 + par
                for o in range(6):
                    col = (4 - kh) * 168 + par * 84 + wo2 * 6 + o
                    for c in range(3):
                        for kw in range(5):
                            wi = wo + kw
                            if wi < 32:
                                t1[c * 32 + wi, col] = w1b[o, c, kh, kw]
                    if kh == 0:
                        t1[96, col] = b1[o]
    return t1.astype(np.float16)


def _build_t1l(w1, b1):
    # lo-pass weights: the fp16 Toeplitz scaled by 2^-11 (exact fp16 normals).
    # Row 96 on kh=0 columns carries the bias residual (b1 - fp16(b1)) * 2^11
    # so the bias gets the same ~22-bit ladder as the data (x-lo row 96 = 1).
    w1b = _binarize(w1)
    b1r = (b1.astype(np.float32) - b1.astype(np.float16).astype(np.float32)) * 2048.0
    t1 = np.zeros((97, 5 * 168), np.float32)
    for kh in range(5):
        for par in range(2):
            for wo2 in range(14):
                wo = 2 * wo2 + par
                for o in range(6):
                    col = (4 - kh) * 168 + par * 84 + wo2 * 6 + o
                    for c in range(3):
                        for kw in range(5):
                            wi = wo + kw
                            if wi < 32:
                                t1[c * 32 + wi, col] = w1b[o, c, kh, kw] * 2.0**-11
                    if kh == 0:
                        # x-lo row 96 is 2^-11, so this lands as b1r * 2^-11
                        # = b1 - fp16(b1) with all-normal fp16 operands.
                        t1[96, col] = b1r[o]
    return t1.astype(np.float16)


def _build_t2(w2):
    # DR pairs: t2dr[w2*6+c, s, (ks*2+wop)*96 + wo2*16+o] = w2b[o,c,2ks+s,kw]
    # tap4:     t24 [w2*6+c, wop*96 + wo2*16+o] = w2b[o,c,4,kw]
    w2b = _binarize(w2)  # [16,6,5,5]
    t2dr = np.zeros((84, 2, 4 * 96), np.float32)
    t24 = np.zeros((84, 2 * 96), np.float32)
    for wop in range(2):
        for wo2 in range(5):
            wo = 2 * wo2 + wop
            for o in range(16):
                for c in range(6):
                    for kw in range(5):
                        w2i = wo + kw
                        if w2i >= 14:
                            continue
                        row = w2i * 6 + c
                        for ks in range(2):
                            for s in range(2):
                                t2dr[row, s, (ks * 2 + wop) * 96 + wo2 * 16 + o] = w2b[
                                    o, c, 2 * ks + s, kw
                                ]
                        t24[row, wop * 96 + wo2 * 16 + o] = w2b[o, c, 4, kw]
    return (
        np.ascontiguousarray(t2dr.reshape(84, 2 * 384)).astype(F8E4),
        t24.astype(F8E4),
    )


def _build_f1(wf1):
    # DR pairs: f1dr[w*16+o, s, h5p*128 + f] = wf1b[f, o*25+(2*h5p+s)*5+w]
    # tap4:     f14 [w*16+o, f] = wf1b[f, o*25+20+w]
    wf1b = _binarize(wf1)  # [120, 400]
    f1dr = np.zeros((80, 2, 2 * 128), np.float32)
    f14 = np.zeros((80, 120), np.float32)
    for w in range(5):
        for o in range(16):
            row = w * 16 + o
            for h5p in range(2):
                for s in range(2):
                    f1dr[row, s, h5p * 128 : h5p * 128 + 120] = wf1b[
                        :, o * 25 + (2 * h5p + s) * 5 + w
                    ]
            f14[row, :] = wf1b[:, o * 25 + 20 + w]
    return (
        np.ascontiguousarray(f1dr.reshape(80, 2 * 256)).astype(F8E4),
        f14.astype(F8E4),
    )


_CACHE = {}


def _get_nc():
    if "nc" in _CACHE:
        return _CACHE["nc"]
    import concourse.bacc as bacc
    import concourse.mybir as mybir
    import concourse.tile as tile

    f32 = mybir.dt.float32
    f16 = mybir.dt.float16
    bf16 = mybir.dt.bfloat16
    f8e4 = mybir.dt.float8e4
    DR = mybir.MatmulPerfMode.DoubleRow

    nc = bacc.Bacc()
    xh_d = nc.dram_tensor("xh", [97, 32 * N], f16, kind="ExternalInput")
    xl_d = nc.dram_tensor("xl", [97, 32 * N], f16, kind="ExternalInput")
    t1h_d = nc.dram_tensor("t1h", [97, 840], f16, kind="ExternalInput")
    t1l_d = nc.dram_tensor("t1l", [97, 840], f16, kind="ExternalInput")
    id_d = nc.dram_tensor("idm", [128, 128], bf16, kind="ExternalInput")
    t2dr_d = nc.dram_tensor("t2dr", [84, 768], f8e4, kind="ExternalInput")
    t24_d = nc.dram_tensor("t24", [84, 192], f8e4, kind="ExternalInput")
    f1dr_d = nc.dram_tensor("f1dr", [80, 512], f8e4, kind="ExternalInput")
    f14_d = nc.dram_tensor("f14", [80, 120], f8e4, kind="ExternalInput")
    f2_d = nc.dram_tensor("f2", [120, 84], bf16, kind="ExternalInput")
    f3_d = nc.dram_tensor("f3", [84, 10], bf16, kind="ExternalInput")
    b2_d = nc.dram_tensor("b2v", [80, 1], f32, kind="ExternalInput")
    bf1_d = nc.dram_tensor("bf1v", [120, 1], f32, kind="ExternalInput")
    bf2_d = nc.dram_tensor("bf2v", [84, 1], f32, kind="ExternalInput")
    bf3_d = nc.dram_tensor("bf3v", [10, 1], f32, kind="ExternalInput")
    out_d = nc.dram_tensor("out", [10, N], f32, kind="ExternalOutput")

    with tile.TileContext(nc) as tc:
        with (
            tc.tile_pool(name="xtp", bufs=1) as xtp,
            tc.tile_pool(name="wts", bufs=1) as wts,
            tc.tile_pool(name="acts", bufs=1) as acts,
            tc.tile_pool(name="ev", bufs=4) as ev,
            tc.tile_pool(name="ps", bufs=1, space="PSUM") as ps,
        ):
            # ---- DMA. sync ring: t1h then the xh tiles; scalar ring: t1l
            # then the xl tiles. A matmul's single wait slot on its x tile
            # transitively covers the same-queue weight DMAs. Each x tile is
            # two DMAs: rows 0:96 (96 divides by 16 -> descriptors spray
            # across all 16 SDMA engines; a 97-row DMA lands on ONE engine)
            # plus the constant ladder row 96 as a tiny flat transfer.
            # Readers wait per-TILE, so tile 0 is ready ~8us in instead of
            # after the whole 12.6MB stream.
            t1hs = wts.tile([97, 840], f16, tag="t1h")
            nc.sync.dma_start(out=t1hs[0:96, :], in_=t1h_d[0:96, :])
            nc.sync.dma_start(out=t1hs[96:97, :], in_=t1h_d[96:97, :])
            t1ls = wts.tile([97, 840], f16, tag="t1l")
            nc.scalar.dma_start(out=t1ls[0:96, :], in_=t1l_d[0:96, :])
            nc.scalar.dma_start(out=t1ls[96:97, :], in_=t1l_d[96:97, :])

            ids = wts.tile([128, 128], bf16, tag="idm")
            xhs, xls = [], []
            for t in range(NT):
                xhs.append(xtp.tile([97, 32 * TI], f16, tag=f"xh{t}", name=f"xh{t}"))
                xls.append(xtp.tile([97, 32 * TI], f16, tag=f"xl{t}", name=f"xl{t}"))
            for t in range(NT):
                sl = slice(t * 32 * TI, (t + 1) * 32 * TI)
                nc.sync.dma_start(out=xhs[t][96:97, :], in_=xh_d[96:97, sl])
                nc.sync.dma_start(out=xhs[t][0:96, :], in_=xh_d[0:96, sl])
                nc.scalar.dma_start(out=xls[t][96:97, :], in_=xl_d[96:97, sl])
                nc.scalar.dma_start(out=xls[t][0:96, :], in_=xl_d[0:96, sl])
                if t == 0:
                    # identity for the transposes - needed from tile 0's
                    # first pool (~14us in), after tile 0's data
                    nc.sync.dma_start(out=ids, in_=id_d[:, :])

            # conv2/fc weights+biases ride the GPSIMD SWDGE ring.
            t2drs = wts.tile([84, 768], f8e4, tag="t2dr")
            nc.gpsimd.dma_start(out=t2drs, in_=t2dr_d[:, :])
            t24s = wts.tile([84, 192], f8e4, tag="t24")
            nc.gpsimd.dma_start(out=t24s, in_=t24_d[:, :])
            f1drs = wts.tile([80, 512], f8e4, tag="f1dr")
            nc.gpsimd.dma_start(out=f1drs, in_=f1dr_d[:, :])
            f14s = wts.tile([80, 120], f8e4, tag="f14")
            nc.gpsimd.dma_start(out=f14s, in_=f14_d[:, :])
            f2s = wts.tile([120, 84], bf16, tag="f2")
            nc.gpsimd.dma_start(out=f2s, in_=f2_d[:, :])
            f3s = wts.tile([84, 10], bf16, tag="f3")
            nc.gpsimd.dma_start(out=f3s, in_=f3_d[:, :])
            b2s = wts.tile([80, 1], f32, tag="b2")
            nc.gpsimd.dma_start(out=b2s, in_=b2_d[:, :])
            bf1s = wts.tile([120, 1], f32, tag="bf1")
            nc.gpsimd.dma_start(out=bf1s, in_=bf1_d[:, :])
            bf2s = wts.tile([84, 1], f32, tag="bf2")
            nc.gpsimd.dma_start(out=bf2s, in_=bf2_d[:, :])
            bf3s = wts.tile([10, 1], f32, tag="bf3")
            nc.gpsimd.dma_start(out=bf3s, in_=bf3_d[:, :])

            # One consumer-engine 'touch' per DMA'd bias tile (carries the DMA
            # wait so later ops on that engine need no extra wait slot).
            tb2 = wts.tile([80, 1], f32, tag="tb2")
            nc.scalar.copy(tb2, b2s)
            tb3 = wts.tile([120, 1], f32, tag="tb3")
            nc.scalar.copy(tb3, bf1s)
            tb4 = wts.tile([84, 1], f32, tag="tb4")
            nc.scalar.copy(tb4, bf2s)
            tb5 = wts.tile([10, 1], f32, tag="tb5")
            nc.vector.tensor_copy(tb5, bf3s)

            # ---- activations / layouts
            # x2: slot-minor [84, a=2*r+s, n]; the rhs DR pair for conv2 row r
            # is columns (2r, 2r+1) = (pooled r, pooled r+1); pooled row q is
            # written to a = 2q (s0) and a = 2q-1 (s1), an adjacent pair.
            x2dr = acts.tile([84, 28 * N], f8e4, tag="x2dr")
            x2b = x2dr.rearrange("p (a n) -> p a n", a=28)
            x3dr = acts.tile([80, 2 * 3 * N], f8e4, tag="x3dr")
            x3v = x3dr.rearrange("p (s f) -> p s f", s=2)
            x4 = acts.tile([120, N], bf16, tag="x4")
            x5 = acts.tile([84, N], bf16, tag="x5")
            outs = acts.tile([10, N], f32, tag="outs")
            c2stage = acts.tile([80, 2 * NB], f32, tag="c2s")
            f1drv = f1drs.rearrange("p (s f) -> p s f", s=2)

            # PSUM: 5 conv1 chunks (1 bank) + transpose rotator (1) + c2 (2).
            NTP = 8
            tp = ps.tile([84, NTP * 128], bf16, tag="tp", name="tp")

            def conv1_groups(h):
                # t1 columns are kh-DESCENDING (block b holds kh=4-b), so the
                # two middle taps whose ho's form one chunk's (even, odd)
                # pair are a single contiguous 336-col matmul writing the
                # whole chunk. Those taps are always accumulation-middle
                # (never kh=0-start / kh=4-stop), so flags stay on singles.
                kset = [kh for kh in range(5) if 0 <= h - kh <= 27]
                pair = (2, 1) if h % 2 == 0 else (3, 2)
                groups = []
                if pair[0] in kset and pair[1] in kset:
                    b0 = 4 - pair[0]
                    groups.append((slice(b0 * 168, (b0 + 2) * 168),
                                   (h - pair[0]) >> 1, slice(0, 336), None))
                    kset = [kh for kh in kset if kh not in pair]
                for kh in kset:
                    ho = h - kh
                    b = 4 - kh
                    groups.append((slice(b * 168, (b + 1) * 168), ho >> 1,
                                   slice((ho & 1) * 168, (ho & 1) * 168 + 168), kh))
                return groups

            def conv1_h(t, h, chunks, passes=("hi", "lo")):
                # PSUM accumulation groups are bank-granular (2KB zero
                # regions): only the bank's FIRST matmul (even ho, hi pass,
                # kh=0) starts it - the odd-ho half then overwrites its
                # pending-zero bytes on first touch - and only the bank's
                # LAST matmul (odd ho, lo pass, kh=4) stops it.
                lhs_h = xhs[t][:, h * TI : (h + 1) * TI]
                lhs_l = xls[t][:, h * TI : (h + 1) * TI]
                for p in passes:
                    for wsl, ci, osl, kh in conv1_groups(h):
                        if p == "hi":
                            nc.tensor.matmul(
                                chunks[ci][:, osl], lhs_h, t1hs[:, wsl],
                                start=(kh == 0 and (h & 1) == 0), stop=False,
                            )
                        else:
                            nc.tensor.matmul(
                                chunks[ci][:, osl], lhs_l, t1ls[:, wsl],
                                start=False, stop=(kh == 4 and (h & 1) == 1),
                            )

            def pool_sign(t, ho2, chunks):
                # 4-way max: ho-pair = chunk col-halves (slots), wo-pair =
                # par halves within a slot -> [128, 84, (s p)] reduce X.
                e = ev.tile([128, 84], bf16, tag="e", bufs=16, name="e")
                nc.vector.reduce_max(
                    e,
                    chunks[ho2].rearrange("q (s p n) -> q n (s p)", s=2, p=2),
                    axis=mybir.AxisListType.X,
                )
                tps = tp[:, (ho2 % NTP) * 128 : (ho2 % NTP) * 128 + 128]
                nc.tensor.transpose(tps, e, ids)
                lo = max(0, 2 * ho2 - 1)
                outp = x2b[:, lo : 2 * ho2 + 1, t * TI : (t + 1) * TI]
                inp = tps.unsqueeze(1)
                if 2 * ho2 + 1 - lo == 2:
                    inp = inp.broadcast_to([84, 2, TI])
                nc.scalar.sign(outp, inp)

            def conv2_phase(c, nb, hop, wop):
                # one conv2 output row hb = 2c+hop, one wo parity, NB imgs;
                # [80, NB] f32 = exactly one PSUM bank; bufs=2 so the next
                # phase's matmuls overlap this phase's stage-copy/pools.
                hb = 2 * c + hop
                nbsl = slice(nb * NB, nb * NB + NB)
                c2t = ps.tile([80, NB], f32, tag="c2", bufs=2, name="c2t")
                for ks in range(2):
                    lhs = t2drs.rearrange("p (s f) -> p s f", s=2)[
                        :, :, (ks * 2 + wop) * 96 : (ks * 2 + wop) * 96 + 80
                    ]
                    r = hb + 2 * ks
                    nc.tensor.matmul(
                        c2t, lhs, x2b[:, 2 * r : 2 * r + 2, nbsl],
                        start=(ks == 0), stop=False, perf_mode=DR,
                    )
                nc.tensor.matmul(
                    c2t, t24s[:, wop * 96 : wop * 96 + 80],
                    x2b[:, 2 * (hb + 4), nbsl], start=False, stop=True,
                )
                return c2t

            def conv2_a(c, nb):
                for wop in range(2):
                    c2t = conv2_phase(c, nb, 0, wop)
                    nc.scalar.copy(c2stage[:, wop * NB : wop * NB + NB], c2t)

            def conv2_b(c, nb):
                ms = []
                for wop in range(2):
                    c2t = conv2_phase(c, nb, 1, wop)
                    m = ev.tile([80, NB], f32, tag="ev", name="m")
                    nc.vector.tensor_max(
                        m, c2t, c2stage[:, wop * NB : wop * NB + NB]
                    )
                    ms.append(m)
                e2 = ev.tile([80, NB], f32, tag="ev", name="e2")
                nc.vector.tensor_max(e2, ms[0], ms[1])
                nc.scalar.sign(
                    x3v[:, c % 2, (c // 2) * N + nb * NB : (c // 2) * N + nb * NB + NB],
                    e2, bias=b2s,
                )

            def fc_block(nb):
                nbsl = slice(nb * NB, nb * NB + NB)
                p3 = ps.tile([120, NB], f32, tag="c2", bufs=2, name="p3")
                for h5p in range(2):
                    nc.tensor.matmul(
                        p3, f1drv[:, :, h5p * 128 : h5p * 128 + 120],
                        x3v[:, :, h5p * N + nb * NB : h5p * N + nb * NB + NB],
                        start=(h5p == 0), stop=False, perf_mode=DR,
                    )
                nc.tensor.matmul(
                    p3, f14s, x3v[:, 0, 2 * N + nb * NB : 2 * N + nb * NB + NB],
                    start=False, stop=True,
                )
                nc.scalar.sign(x4[:, nbsl], p3, bias=bf1s)

                p4 = ps.tile([84, NB], f32, tag="c2", bufs=2, name="p4")
                nc.tensor.matmul(p4, f2s, x4[:, nbsl], start=True, stop=True)
                nc.scalar.sign(x5[:, nbsl], p4, bias=bf2s)

                p5 = ps.tile([10, NB], f32, tag="c2", bufs=2, name="p5")
                nc.tensor.matmul(p5, f3s, x5[:, nbsl], start=True, stop=True)
                nc.vector.tensor_scalar_add(outs[:, nbsl], p5, bf3s)
                nc.sync.dma_start(out=out_d[:, nbsl], in_=outs[:, nbsl])

            # conv2/fc interleave schedule: (t, h) -> list of emitter thunks.
            # nb=0 spreads through tiles 4-6, nb=1 through tile 7 as its
            # pooled rows appear; nb=1's last block + fc trail at the end.
            def emit_a(c, nb):
                return lambda: conv2_a(c, nb)

            def emit_b(c, nb):
                return lambda: conv2_b(c, nb)

            sched = {}
            for c in range(5):
                sched.setdefault((4 + (c // 2), (4 * c) % 8 + 2), []).append(emit_a(c, 0))
                sched.setdefault((4 + (c // 2), (4 * c) % 8 + 4), []).append(emit_b(c, 0))
            sched.setdefault((6, 24), []).append(lambda: fc_block(0))
            for c in range(4):
                sched.setdefault((7, 16 + 4 * c), []).append(emit_a(c, 1))
                sched.setdefault((7, 18 + 4 * c), []).append(emit_b(c, 1))

            # pool chunk a two h-steps after its last matmul (h = 2a+7) so
            # the in-order PE doesn't stall at the transpose waiting on DVE;
            # ho2=13's pool carries into the next tile's h=1 slot.
            prev_carry = []
            for t in range(NT):
                chunks = {}
                carry, prev_carry = prev_carry, []
                for h in range(32):
                    if h % 2 == 0 and h <= 26:
                        chunks[h >> 1] = ps.tile(
                            [128, 336], f32, tag="chunk", bufs=5, name="ck"
                        )
                    if t == 0 and h < 3:
                        # tile 0 warmup: hi-pass only while the xl stream is
                        # still landing; the lo passes backfill before h+3's
                        # groups (so each chunk's stop stays its last write)
                        conv1_h(t, h, chunks, passes=("hi",))
                    else:
                        if t == 0 and h < 6:
                            conv1_h(t, h - 3, chunks, passes=("lo",))
                        conv1_h(t, h, chunks)
                    if h == 1 and carry:
                        carry.pop(0)()
                    if h >= 7 and h % 2 == 1 and h <= 31:
                        a = (h - 7) // 2
                        pool_sign(t, a, chunks)
                    for fn in sched.get((t, h), []):
                        fn()
                prev_carry.append(
                    (lambda tt, ck: lambda: pool_sign(tt, 13, ck))(t, chunks)
                )
            for fn in prev_carry:  # tile 7's last pool
                fn()
            # tail: last conv2 block of nb=1, then fc
            sched_tail = [emit_a(4, 1), emit_b(4, 1)]
            for fn in sched_tail:
                fn()
            fc_block(1)

    nc.finalize()
    _CACHE["nc"] = nc
    return nc


def _install_ntff_hook():
    """The container's antenv stub lacks axon_hooks; synthesize it and register
    the ctypes-based NTFF profile hook from the axon boot module."""
    if "hook" in _CACHE:
        return
    _CACHE["hook"] = True
    try:
        import types
        import antenv

        if not hasattr(antenv, "axon_hooks"):
            store = {"h": None}
            m = types.ModuleType("antenv.axon_hooks")
            m.set_axon_ntff_profile_hook = lambda h: store.update(h=h)
            m.get_axon_ntff_profile_hook = lambda: store["h"]
            sys.modules["antenv.axon_hooks"] = m
            antenv.axon_hooks = m
            sys.path.insert(0, "/root/.axon_site")
            from trn_agent_boot.trn_boot import _ntff_profile_via_ctypes

            m.set_axon_ntff_profile_hook(
                _ntff_profile_via_ctypes("/opt/axon/libaxon_pjrt.so")
            )
    except Exception as e:  # profiling is best-effort
        print(f"ntff hook install failed: {e}", file=sys.stderr)


def kernel(x, w1, b1, w2, b2, wf1, bf1, wf2, bf2, wf3, bf3):
    nc = _get_nc()
    _install_ntff_hook()
    from concourse import bass_utils

    # host-side relayout: xt[core][c*32+w, (t, h, n)] = x[core*N + t*TI + n, c, h, w]
    xr = np.ascontiguousarray(
        x.reshape(NCORES, NT, TI, 3, 32, 32).transpose(0, 3, 5, 1, 4, 2)
    )  # [cores, 3, 32(w), NT, 32(h), TI]

    xh = np.ones((NCORES, 97, NT, 32, TI), np.float16)
    xh[:, :96] = xr.reshape(NCORES, 96, NT, 32, TI).astype(np.float16)
    xl = np.full((NCORES, 97, NT, 32, TI), 2.0**-11, np.float16)
    xl[:, :96] = (
        (xr.reshape(NCORES, 96, NT, 32, TI) - xh[:, :96].astype(np.float32)) * 2048.0
    ).astype(np.float16)

    t2dr, t24 = _build_t2(w2)
    f1dr, f14 = _build_f1(wf1)
    shared = {
        "t1h": _build_t1h(w1, b1),
        "t1l": _build_t1l(w1, b1),
        "idm": np.eye(128, dtype=BF16),
        "t2dr": t2dr, "t24": t24, "f1dr": f1dr, "f14": f14,
        "f2": np.ascontiguousarray(_binarize(wf2).T).astype(BF16),
        "f3": np.ascontiguousarray(_binarize(wf3).T).astype(BF16),
        "b2v": np.tile(b2.astype(np.float32), 5).reshape(80, 1),
        "bf1v": bf1.astype(np.float32).reshape(120, 1),
        "bf2v": bf2.astype(np.float32).reshape(84, 1),
        "bf3v": bf3.astype(np.float32).reshape(10, 1),
    }
    in_maps = [
        dict(
            shared,
            xh=np.ascontiguousarray(xh[i].reshape(97, 32 * N)),
            xl=np.ascontiguousarray(xl[i].reshape(97, 32 * N)),
        )
        for i in range(NCORES)
    ]

    res = bass_utils.run_bass_kernel_spmd(
        nc, in_maps, core_ids=list(range(NCORES)),
        trace=bool(int(os.environ.get("KERNEL_TRACE", "0"))),
    )
    if res.exec_time_ns is not None:
        print(f"HW exec time: {res.exec_time_ns} ns")
    out = np.stack([r["out"] for r in res.results])  # [8, 10, N]
    return np.ascontiguousarray(out.transpose(0, 2, 1)).reshape(B, 10).astype(np.float32)
